# revision 4
# baseline (speedup 1.0000x reference)
"""bf16 C4 depthwise conv — all-PE dy-contraction kernel.

Each 32-channel group's padded input is staged as xrep[96, 128, 130]:
three row-shifted copies (vertical taps dy=0,1,2) stacked in partition
blocks, loaded with three parity-alternating DMAs on the SP HWDGE ring.
One bf16 stationary S_tj[96, 128] per horizontal tap then contracts all
three vertical taps AND emits all 4 rotations x 32 channels per matmul:
S_tj[(dy, c), (c*4 + r)] = rot90(w[c], r)[dy, tj].  Three matmuls
(tj=0,1,2) accumulate each [128, 4, 128] PSUM strip; two strips share a
[128, 8, 128] PSUM tile whose single drain (ACT/DVE alternating) casts
to bf16.  Outputs are packed partition-interleaved (p = c*4 + r) so
each fused per-rotation 1 MB store on the ACT ring reads a strided
partition set hitting all 16 SDMA engines.
"""

import numpy as np
from contextlib import ExitStack

from concourse import bacc, mybir, tile

B, C, H, W = 16, 192, 128, 128
NCORES = 8
BS = B // NCORES
ROWS = BS * C               # 384 (batch, channel) rows per core
G = 32                      # channels per group
NG = ROWS // G              # 12 groups
NGC = C // G                # 6 unique channel groups (stationaries repeat per batch)
TW = W + 2                  # 130
TH = H + 2                  # 130
SUB = 4                     # matmul rows (one PSUM bank, N=512)
PT = 8                      # psum tile rows (2 banks, one drain)
NSTRIP = H // PT            # 16 psum tiles per group

F32 = mybir.dt.float32
BF16 = mybir.dt.bfloat16

BLK = [0, 2, 1]             # dy -> xrep partition block (parity-alternating loads)


def _build(drain_pat=("act", "dve", "act", "dve", "act", "dve", "act", "act"),
           repeat=1):
    nc = bacc.Bacc("TRN2", target_bir_lowering=False, debug=False, num_devices=NCORES)
    x_d = nc.dram_tensor("x", [ROWS, TH, TW], BF16, kind="ExternalInput").ap()
    st_d = nc.dram_tensor("stat", [96, NGC, 3, 128], BF16, kind="ExternalInput").ap()
    o_d = nc.dram_tensor("out", [BS * 4 * C, H, W], BF16, kind="ExternalOutput").ap()

    with tile.TileContext(nc) as tc, ExitStack() as ctx:
        xpool = ctx.enter_context(tc.tile_pool(name="xrep", bufs=4))
        wpool = ctx.enter_context(tc.tile_pool(name="wst", bufs=1))
        opool = ctx.enter_context(tc.tile_pool(name="osb", bufs=2))
        pspool = ctx.enter_context(tc.tile_pool(name="ps", bufs=4, space="PSUM"))

        st_sb = wpool.tile([96, NGC, 3, 128], BF16, tag="st")
        nc.sync.dma_start(st_sb[:], st_d[:, :, :, :])

        di = 0
        for g in range(NG * repeat):
            g = g % NG
            b, gi = g // NGC, g % NGC
            xr = xpool.tile([96, H, TW], BF16, tag="xr")
            for dy in range(3):
                blk = BLK[dy]
                nc.sync.dma_start(
                    xr[32 * blk : 32 * (blk + 1), :, :],
                    x_d[G * g : G * (g + 1), dy : dy + H, :],
                )
            osb = opool.tile([128, H, W], BF16, tag="osb")
            for s in range(NSTRIP):
                r0 = s * PT
                ps = pspool.tile([128, PT, W], F32, tag="ps")
                for tj in range(3):
                    for m in range(PT // SUB):
                        nc.tensor.matmul(
                            ps[:, m * SUB : (m + 1) * SUB, :],
                            st_sb[:, gi, tj, :],
                            xr[:, r0 + m * SUB : r0 + m * SUB + SUB, tj : tj + W],
                            start=(tj == 0),
                            stop=(tj == 2),
                        )
                eng = drain_pat[di % len(drain_pat)]
                di += 1
                osl = osb[:, r0 : r0 + PT, :]
                if eng == "act":
                    nc.scalar.activation(
                        osl, ps[:], mybir.ActivationFunctionType.Copy
                    )
                else:
                    nc.vector.tensor_copy(osl, ps[:])
            for r in range(4):
                row0 = b * 4 * C + r * C + G * gi
                nc.scalar.dma_start(
                    o_d[row0 : row0 + G, :, :],
                    osb[r::4, :, :],
                )

    nc.compile()
    return nc


def _make_stat(weight):
    bf = mybir.dt.np(BF16)
    wb = weight[:, 0].astype(np.float32)          # [C, 3, 3]
    stat = np.zeros((96, NGC, 3, 128), dtype=np.float32)
    ar = np.arange(G)
    for r in range(4):
        wr = np.rot90(wb, r, axes=(1, 2))         # [C, 3, 3]
        for dy in range(3):
            for tj in range(3):
                vals = wr[:, dy, tj].reshape(NGC, G)   # [NGC, G]
                stat[BLK[dy] * G + ar, :, tj, ar * 4 + r] = vals.T
    return stat.astype(bf)


def make_in_maps(x, weight):
    bf = mybir.dt.np(BF16)
    stat = _make_stat(weight)
    xp = np.zeros((B, C, TH, TW), dtype=bf)
    xp[:, :, 1 : H + 1, 1 : W + 1] = x.astype(bf)
    return [
        {
            "x": np.ascontiguousarray(xp[BS * k : BS * (k + 1)].reshape(ROWS, TH, TW)),
            "stat": stat,
        }
        for k in range(NCORES)
    ]


from concourse.bass_utils import run_bass_kernel_spmd

_NC = None


def _get_nc():
    global _NC
    if _NC is None:
        _NC = _build()
    return _NC


def kernel(x, weight):
    x = np.asarray(x, dtype=np.float32)
    weight = np.asarray(weight, dtype=np.float32)
    in_maps = make_in_maps(x, weight)
    nc = _get_nc()
    res = run_bass_kernel_spmd(nc, in_maps, list(range(NCORES))).results
    out = np.empty((B, 4 * C, H, W), dtype=np.float32)
    for k in range(NCORES):
        out[BS * k : BS * (k + 1)] = (
            res[k]["out"].astype(np.float32).reshape(BS, 4 * C, H, W)
        )
    return out


# revision 5
# speedup vs baseline: 1.3109x; 1.3109x over previous
"""bf16 C4 depthwise conv — all-PE dy-contraction kernel.

Each 32-channel group's padded input is staged as xrep[96, 128, 130]:
three row-shifted copies (vertical taps dy=0,1,2) stacked in partition
blocks, loaded with three parity-alternating DMAs on the SP HWDGE ring.
One bf16 stationary S_tj[96, 128] per horizontal tap then contracts all
three vertical taps AND emits all 4 rotations x 32 channels per matmul:
S_tj[(dy, c), (c*4 + r)] = rot90(w[c], r)[dy, tj].  Three matmuls
(tj=0,1,2) accumulate each [128, 4, 128] PSUM strip; two strips share a
[128, 8, 128] PSUM tile whose single drain (ACT/DVE alternating) casts
to bf16.  Outputs are packed partition-interleaved (p = c*4 + r) so
each fused per-rotation 1 MB store on the ACT ring reads a strided
partition set hitting all 16 SDMA engines.
"""

import numpy as np
from contextlib import ExitStack

from concourse import bacc, mybir, tile

B, C, H, W = 16, 192, 128, 128
NCORES = 8
BS = B // NCORES
ROWS = BS * C               # 384 (batch, channel) rows per core
G = 32                      # channels per group
NG = ROWS // G              # 12 groups
NGC = C // G                # 6 unique channel groups (stationaries repeat per batch)
TW = W + 2                  # 130
TH = H + 2                  # 130
SUB = 4                     # matmul rows (one PSUM bank, N=512)
PT = 8                      # psum tile rows (2 banks, one drain)
NSTRIP = H // PT            # 16 psum tiles per group

F32 = mybir.dt.float32
BF16 = mybir.dt.bfloat16

BLK = [0, 2, 1]             # dy -> xrep partition block (parity-alternating loads)


def _build(drain_pat=("act", "dve", "act", "dve", "act", "dve", "act", "act"),
           repeat=1):
    nc = bacc.Bacc("TRN2", target_bir_lowering=False, debug=False, num_devices=NCORES)
    x_d = nc.dram_tensor("x", [ROWS, TH, TW], BF16, kind="ExternalInput").ap()
    st_d = nc.dram_tensor("stat", [96, NGC, 3, 128], BF16, kind="ExternalInput").ap()
    o_d = nc.dram_tensor("out", [BS * 4 * C, H, W], BF16, kind="ExternalOutput").ap()

    with tile.TileContext(nc) as tc, ExitStack() as ctx:
        xpool = ctx.enter_context(tc.tile_pool(name="xrep", bufs=3))
        wpool = ctx.enter_context(tc.tile_pool(name="wst", bufs=1))
        opool = ctx.enter_context(tc.tile_pool(name="osb", bufs=2))
        pspool = ctx.enter_context(tc.tile_pool(name="ps", bufs=4, space="PSUM"))

        st_sb = wpool.tile([96, NGC, 3, 128], BF16, tag="st")
        nc.sync.dma_start(st_sb[:], st_d[:, :, :, :])

        di = 0
        for g in range(NG * repeat):
            g = g % NG
            b, gi = g // NGC, g % NGC
            xr = xpool.tile([96, H, TW], BF16, tag="xr")
            for dy in range(3):
                blk = BLK[dy]
                nc.sync.dma_start(
                    xr[32 * blk : 32 * (blk + 1), :, :],
                    x_d[G * g : G * (g + 1), dy : dy + H, :],
                )
            osb = opool.tile([128, H, W], BF16, tag="osb")
            for s in range(NSTRIP):
                r0 = s * PT
                ps = pspool.tile([128, PT, W], F32, tag="ps")
                for tj in range(3):
                    for m in range(PT // SUB):
                        nc.tensor.matmul(
                            ps[:, m * SUB : (m + 1) * SUB, :],
                            st_sb[:, gi, tj, :],
                            xr[:, r0 + m * SUB : r0 + m * SUB + SUB, tj : tj + W],
                            start=(tj == 0),
                            stop=(tj == 2),
                        )
                eng = drain_pat[di % len(drain_pat)]
                di += 1
                osl = osb[:, r0 : r0 + PT, :]
                if eng == "act":
                    nc.scalar.activation(
                        osl, ps[:], mybir.ActivationFunctionType.Copy
                    )
                else:
                    nc.vector.tensor_copy(osl, ps[:])
            for r in range(4):
                row0 = b * 4 * C + r * C + G * gi
                nc.scalar.dma_start(
                    o_d[row0 : row0 + G, :, :],
                    osb[r::4, :, :],
                )

    nc.compile()
    return nc


def _make_stat(weight):
    bf = mybir.dt.np(BF16)
    wb = weight[:, 0].astype(np.float32)          # [C, 3, 3]
    stat = np.zeros((96, NGC, 3, 128), dtype=np.float32)
    ar = np.arange(G)
    for r in range(4):
        wr = np.rot90(wb, r, axes=(1, 2))         # [C, 3, 3]
        for dy in range(3):
            for tj in range(3):
                vals = wr[:, dy, tj].reshape(NGC, G)   # [NGC, G]
                stat[BLK[dy] * G + ar, :, tj, ar * 4 + r] = vals.T
    return stat.astype(bf)


def make_in_maps(x, weight):
    bf = mybir.dt.np(BF16)
    stat = _make_stat(weight)
    xp = np.zeros((B, C, TH, TW), dtype=bf)
    xp[:, :, 1 : H + 1, 1 : W + 1] = x.astype(bf)
    return [
        {
            "x": np.ascontiguousarray(xp[BS * k : BS * (k + 1)].reshape(ROWS, TH, TW)),
            "stat": stat,
        }
        for k in range(NCORES)
    ]


from concourse.bass_utils import run_bass_kernel_spmd

_NC = None


def _get_nc():
    global _NC
    if _NC is None:
        _NC = _build()
    return _NC


def kernel(x, weight):
    x = np.asarray(x, dtype=np.float32)
    weight = np.asarray(weight, dtype=np.float32)
    in_maps = make_in_maps(x, weight)
    nc = _get_nc()
    res = run_bass_kernel_spmd(nc, in_maps, list(range(NCORES))).results
    out = np.empty((B, 4 * C, H, W), dtype=np.float32)
    for k in range(NCORES):
        out[BS * k : BS * (k + 1)] = (
            res[k]["out"].astype(np.float32).reshape(BS, 4 * C, H, W)
        )
    return out
